# revision 1
# baseline (speedup 1.0000x reference)
"""BertSelfAttention on 8 Trainium2 NeuronCores.

Sharding: core c handles batch b = c // 4 and head group g = c % 4 (4 of the 16
heads, i.e. columns [256*g, 256*(g+1)) of the QKV projections). No cross-core
communication.

Per-core device algorithm (everything f32r on the PE, fp32 accumulation):
  qT, kT  [256, 2048] = W{q,k}_c @ x_b.T   (+ per-partition bias)
  v       [2048, 4, 65]: natural-layout V per head, scaled by exp(mask) rows,
          with a 65th all-exp(mask) column (softmax denominator rides the PV
          matmul).
  per head, per 256-wide query chunk:
     scoresT [sk, sq] = kT_h.T-pair-packed K=64 matmuls (two heads overlap in
              disjoint PE row groups)
     E = exp(scoresT / 8)    (ACT, PSUM -> SBUF f32r, 4-sk-tile groups)
     ctx_aug [65, sq] += v_aug_h[sk_tile].T @ E        (16 sk tiles)
  out ctxT [4, 65, 2048] -> host divides rows 0..63 by row 64, adds bv,
  transposes into [B, S, D].
"""
import numpy as np

import concourse.bacc as bacc
import concourse.tile as tile
import concourse.mybir as mybir
from concourse.bass_utils import run_bass_kernel_spmd

dt = mybir.dt
AF = mybir.ActivationFunctionType

B, S, D, H = 2, 2048, 1024, 16
HD = 64
NH = 4            # heads per core
GW = NH * HD      # head-group width = 256
P = 128
DKT = D // P      # 8 k-tiles for projections
SKT = S // P      # 16 key tiles
CW = 256          # query chunk width
NCH = S // CW     # 8 query chunks
VW = HD + 1       # v + ones column

_CACHE = {}


def _build():
    nc = bacc.Bacc(None, target_bir_lowering=False)

    xT = nc.dram_tensor("xT", [D, S], dt.float32, kind="ExternalInput")
    wT = nc.dram_tensor("wT", [3, D, GW], dt.float32, kind="ExternalInput")
    bqk = nc.dram_tensor("bqk", [2, GW], dt.float32, kind="ExternalInput")
    mask = nc.dram_tensor("mask", [S], dt.float32, kind="ExternalInput")
    ctxT = nc.dram_tensor("ctxT", [NH, VW, S], dt.float32, kind="ExternalOutput")

    with tile.TileContext(nc) as tc:
        with tc.tile_pool(name="const", bufs=1) as const, \
             tc.tile_pool(name="epool", bufs=3) as epool, \
             tc.tile_pool(name="opool", bufs=4) as opool, \
             tc.tile_pool(name="spsum", bufs=3, space="PSUM") as spsum, \
             tc.tile_pool(name="cpsum", bufs=2, space="PSUM") as cpsum:

            # ---- loads ----
            xT_sb = const.tile([P, DKT, S], dt.float32r)
            nc.gpsimd.dma_start(out=xT_sb[:], in_=xT.rearrange("(t p) s -> p t s", p=P))
            wT_sb = const.tile([P, 3, DKT, GW], dt.float32r)
            nc.gpsimd.dma_start(out=wT_sb[:], in_=wT.rearrange("w (t p) n -> p w t n", p=P))
            bqk_sb = const.tile([P, 2, 2], dt.float32)
            nc.gpsimd.dma_start(out=bqk_sb[:], in_=bqk.rearrange("w (m p) -> p w m", p=P))
            mask_sb = const.tile([P, SKT], dt.float32)
            nc.gpsimd.dma_start(out=mask_sb[:], in_=mask.rearrange("(t p) -> p t", p=P))

            # exp(mask), fp32 (used as DVE per-partition scalar + ones cols)
            em_sb = const.tile([P, SKT], dt.float32)
            nc.scalar.activation(em_sb[:], mask_sb[:], AF.Exp)

            qT_sb = const.tile([P, 2, S], dt.float32r)   # [n-tile, 2 Mtiles, s]
            kT_sb = const.tile([P, 2, S], dt.float32r)
            v_sb = const.tile([P, NH, SKT, VW], dt.float32r)

            # ---- projections ----
            def proj_qk(w_idx, m, dst):
                for c in range(S // 512):
                    ps = spsum.tile([P, 1024], dt.float32, tag="sc")
                    for kd in range(DKT):
                        nc.tensor.matmul(
                            ps[:, :512],
                            wT_sb[:, w_idx, kd, m * P:(m + 1) * P],
                            xT_sb[:, kd, c * 512:(c + 1) * 512],
                            start=(kd == 0), stop=(kd == DKT - 1),
                        )
                    nc.vector.tensor_scalar_add(
                        dst[:, m, c * 512:(c + 1) * 512], ps[:, :512],
                        bqk_sb[:, w_idx, m:m + 1],
                    )

            # heads 0,1 first so attention on pair 0 can start early
            proj_qk(0, 0, qT_sb)
            proj_qk(1, 0, kT_sb)

            # v natural layout, all 4 heads; rows scaled by exp(mask)
            for t in range(SKT):
                ps = spsum.tile([P, 1024], dt.float32, tag="sc")
                for kd in range(DKT):
                    nc.tensor.matmul(
                        ps[:, :GW],
                        xT_sb[:, kd, t * P:(t + 1) * P],
                        wT_sb[:, 2, kd, :],
                        start=(kd == 0), stop=(kd == DKT - 1),
                    )
                for h in range(NH):
                    nc.vector.tensor_scalar_mul(
                        v_sb[:, h, t, 0:HD], ps[:, h * HD:(h + 1) * HD],
                        em_sb[:, t:t + 1],
                    )
            # ones columns = exp(mask)
            for h in range(NH):
                nc.vector.tensor_copy(v_sb[:, h, :, HD], em_sb[:])

            proj_qk(0, 1, qT_sb)
            proj_qk(1, 1, kT_sb)

            # ---- attention, head pairs ----
            for p in range(2):           # head pair: heads (2p, 2p+1)
                for ch in range(NCH):
                    sq = slice(ch * CW, (ch + 1) * CW)
                    ctx0 = cpsum.tile([VW, CW], dt.float32, tag="ctx")
                    ctx1 = cpsum.tile([VW, CW], dt.float32, tag="ctx")
                    for g in range(SKT // 4):
                        psA = spsum.tile([P, 1024], dt.float32, tag="sc")
                        psB = spsum.tile([P, 1024], dt.float32, tag="sc")
                        for j in range(4):
                            t = 4 * g + j
                            js = slice(j * CW, (j + 1) * CW)
                            # row-packed pair: PE rows 0-63 and 64-127
                            nc.tensor.matmul(
                                psA[:, js],
                                kT_sb[0:HD, p, t * P:(t + 1) * P],
                                qT_sb[0:HD, p, sq],
                                start=True, stop=True,
                            )
                            nc.tensor.matmul(
                                psB[:, js],
                                kT_sb[HD:P, p, t * P:(t + 1) * P],
                                qT_sb[HD:P, p, sq],
                                start=True, stop=True,
                            )
                        eA = epool.tile([P, 1024], dt.float32r, tag="eA")
                        eB = epool.tile([P, 1024], dt.float32r, tag="eB")
                        nc.scalar.activation(eA[:], psA[:], AF.Exp, scale=0.125)
                        nc.scalar.activation(eB[:], psB[:], AF.Exp, scale=0.125)
                        for j in range(4):
                            t = 4 * g + j
                            js = slice(j * CW, (j + 1) * CW)
                            nc.tensor.matmul(
                                ctx0[:], v_sb[:, 2 * p, t, :], eA[:, js],
                                start=(t == 0), stop=(t == SKT - 1),
                            )
                            nc.tensor.matmul(
                                ctx1[:], v_sb[:, 2 * p + 1, t, :], eB[:, js],
                                start=(t == 0), stop=(t == SKT - 1),
                            )
                    o0 = opool.tile([VW, CW], dt.float32, tag="o")
                    o1 = opool.tile([VW, CW], dt.float32, tag="o")
                    nc.vector.tensor_copy(o0[:], ctx0[:])
                    nc.vector.tensor_copy(o1[:], ctx1[:])
                    nc.sync.dma_start(out=ctxT[2 * p, :, sq], in_=o0[:])
                    nc.sync.dma_start(out=ctxT[2 * p + 1, :, sq], in_=o1[:])

    nc.compile()
    return nc


def kernel(hidden_states, attention_mask, Wq, bq, Wk, bk, Wv, bv):
    hidden_states = np.asarray(hidden_states, dtype=np.float32)
    attention_mask = np.asarray(attention_mask, dtype=np.float32)
    Wq = np.asarray(Wq, dtype=np.float32)
    Wk = np.asarray(Wk, dtype=np.float32)
    Wv = np.asarray(Wv, dtype=np.float32)
    bq = np.asarray(bq, dtype=np.float32)
    bk = np.asarray(bk, dtype=np.float32)
    bv = np.asarray(bv, dtype=np.float32)

    if "nc" not in _CACHE:
        _CACHE["nc"] = _build()
    nc = _CACHE["nc"]

    xTs = [np.ascontiguousarray(hidden_states[b].T) for b in range(B)]
    in_maps = []
    for c in range(8):
        b, g = c // 4, c % 4
        cols = slice(g * GW, (g + 1) * GW)
        in_maps.append({
            "xT": xTs[b],
            "wT": np.ascontiguousarray(
                np.stack([Wq[cols].T, Wk[cols].T, Wv[cols].T])),
            "bqk": np.ascontiguousarray(np.stack([bq[cols], bk[cols]])),
            "mask": np.ascontiguousarray(attention_mask[b, 0, 0, :]),
        })

    res = run_bass_kernel_spmd(nc, in_maps, list(range(8)))

    out = np.empty((B, S, D), dtype=np.float32)
    for c in range(8):
        b, g = c // 4, c % 4
        r = res.results[c]["ctxT"]                     # [NH, VW, S]
        ctx = r[:, :HD, :] / r[:, HD:HD + 1, :]        # [NH, HD, S]
        ctx = ctx + bv[g * GW:(g + 1) * GW].reshape(NH, HD)[:, :, None]
        out[b, :, g * GW:(g + 1) * GW] = ctx.transpose(2, 0, 1).reshape(S, GW)
    return out


# revision 24
# speedup vs baseline: 22.6178x; 22.6178x over previous
"""BertSelfAttention on 8 Trainium2 NeuronCores.

Sharding: core c handles batch b = c // 4 and head group g = c % 4 (4 of the 16
heads, i.e. columns [256*g, 256*(g+1)) of the QKV projections). No cross-core
communication.

Per-core device algorithm (everything f32r on the PE, fp32 accumulation):
  qT, kT  [256, 2048] = W{q,k}_c @ x_b.T   (+ per-partition bias)
  v       [2048, 4, 65]: natural-layout V per head, scaled by exp(mask) rows,
          with a 65th all-exp(mask) column (softmax denominator rides the PV
          matmul).
  per head, per 256-wide query chunk:
     scoresT [sk, sq] = kT_h.T-pair-packed K=64 matmuls (two heads overlap in
              disjoint PE row groups)
     E = exp(scoresT / 8)    (ACT, PSUM -> SBUF f32r, 4-sk-tile groups)
     ctx_aug [65, sq] += v_aug_h[sk_tile].T @ E        (16 sk tiles)
  out ctxT [4, 65, 2048] -> host divides rows 0..63 by row 64, adds bv,
  transposes into [B, S, D].
"""
import numpy as np

import concourse.bacc as bacc
import concourse.tile as tile
import concourse.mybir as mybir
from concourse.bass_utils import run_bass_kernel_spmd

dt = mybir.dt
AF = mybir.ActivationFunctionType

B, S, D, H = 2, 2048, 1024, 16
HD = 64
NH = 4            # heads per core
GW = NH * HD      # head-group width = 256
P = 128
DKT = D // P      # 8 k-tiles for projections
SKT = S // P      # 16 key tiles
CW = 256          # query chunk width
NCH = S // CW     # 8 query chunks
VW = HD + 1       # v + ones column

_CACHE = {}


def _build(reps=1):
    nc = bacc.Bacc(None, target_bir_lowering=False)

    # f32r declarations: bitwise fp32; the PE rounds internally (verified
    # identical to an explicit cast-DMA), so plain HWDGE DMAs suffice.
    xT = nc.dram_tensor("xT", [D, S], dt.float32r, kind="ExternalInput")
    wT = nc.dram_tensor("wT", [3, D, GW], dt.float32r, kind="ExternalInput")
    bqk = nc.dram_tensor("bqk", [2, GW], dt.float32, kind="ExternalInput")
    mask = nc.dram_tensor("mask", [S, NH], dt.float32, kind="ExternalInput")
    ctxT = nc.dram_tensor("ctxT", [NH, VW, S], dt.float32, kind="ExternalOutput")

    with tile.TileContext(nc) as tc:
        with tc.tile_pool(name="const", bufs=1) as const, \
             tc.tile_pool(name="epool", bufs=4) as epool, \
             tc.tile_pool(name="opool", bufs=4) as opool, \
             tc.tile_pool(name="spsum", bufs=3, space="PSUM") as spsum, \
             tc.tile_pool(name="cpsum", bufs=2, space="PSUM") as cpsum:

            # ---- loads (xT split by s-chunk so the first projection groups
            # only wait on their own 2MB slice) ----
            wT_r = wT.rearrange("w (t p) n -> p w t n", p=P)
            wT_sb = const.tile([P, 3, DKT, GW], dt.float32r)
            for w in range(3):
                nc.sync.dma_start(out=wT_sb[:, w, :, :], in_=wT_r[:, w, :, :])
            xT_r = xT.rearrange("(t p) s -> p t s", p=P)
            xT_sb = const.tile([P, DKT, S], dt.float32r)
            for c in range(4):
                cs = slice(c * 512, (c + 1) * 512)
                nc.sync.dma_start(out=xT_sb[:, :, cs], in_=xT_r[:, :, cs])
            bqk_sb = const.tile([P, 2, 2], dt.float32)
            nc.sync.dma_start(out=bqk_sb[:], in_=bqk.rearrange("w (m p) -> p w m", p=P))
            mask_sb = const.tile([P, SKT, NH], dt.float32)
            nc.sync.dma_start(out=mask_sb[:], in_=mask.rearrange("(t p) h -> p t h", p=P))

            # exp(mask), fp32 (used as DVE per-partition scalar + ones cols)
            em_sb = const.tile([P, SKT, NH], dt.float32)
            nc.scalar.activation(em_sb[:], mask_sb[:], AF.Exp)

            for _rep in range(reps):
                _emit_body(nc, const, epool, opool, spsum, cpsum,
                           xT_sb, wT_sb, bqk_sb, em_sb, ctxT)

    nc.compile()
    return nc


def _emit_body(nc, const, epool, opool, spsum, cpsum,
               xT_sb, wT_sb, bqk_sb, em_sb, ctxT):
    # per-chunk projection tiles (fine-grained deps so attention can begin
    # while later projection chunks are still computing)
    qT = [[None] * 4 for _ in range(2)]   # [m][c] -> [128, 512] (s-chunk c)
    kT = [[None] * 4 for _ in range(2)]
    # v tiles pre-created: PV emission can reference them before the aux
    # queue emits their producing instructions (scheduler orders by deps)
    vt = [const.tile([P, NH, VW], dt.float32r, tag=f"v{t}", name=f"v{t}")
          for t in range(SKT)]

    def proj_qk_group(w_idx, m, c):
        ps = spsum.tile([P, 1024], dt.float32, tag="sc")
        for kd in range(DKT):
            nc.tensor.matmul(
                ps[:, :512],
                wT_sb[:, w_idx, kd, m * P:(m + 1) * P],
                xT_sb[:, kd, c * 512:(c + 1) * 512],
                start=(kd == 0), stop=(kd == DKT - 1),
            )
        dst = const.tile([P, 512], dt.float32r, tag=f"{'qk'[w_idx]}{m}{c}")
        nc.vector.tensor_scalar_add(dst[:], ps[:, :512], bqk_sb[:, w_idx, m:m + 1])
        (qT if w_idx == 0 else kT)[m][c] = dst

    def proj_v_group(t):
        ps = spsum.tile([P, 1024], dt.float32, tag="sc")
        for kd in range(DKT):
            nc.tensor.matmul(
                ps[:, :GW],
                xT_sb[:, kd, t * P:(t + 1) * P],
                wT_sb[:, 2, kd, :],
                start=(kd == 0), stop=(kd == DKT - 1),
            )
        v = vt[t]
        for h in range(NH):
            nc.vector.tensor_scalar_mul(
                v[:, h, 0:HD], ps[:, h * HD:(h + 1) * HD], em_sb[:, t, 0:1])
        nc.vector.tensor_copy(v[:, :, HD], em_sb[:, t, :])

    # prelude: only what pair-0 chunk-0 group-0 needs (q0 c0, k0 c0, v t0..3).
    # Everything else is injected just-in-time into early attention iterations
    # — always in program order before any consumer (Tile's dependency model
    # is program-order-imperative).
    proj_qk_group(0, 0, 0)
    proj_qk_group(1, 0, 0)
    for t in range(4):
        proj_v_group(t)

    # aux queue: lists of groups to inject per attention iteration
    aux = [[("k", 0, 1), ("v", 4), ("v", 5)],
           [("k", 0, 2), ("v", 6), ("v", 7)],
           [("k", 0, 3), ("v", 8), ("v", 9)],
           [("v", 10), ("v", 11), ("v", 12)],
           [("v", 13), ("v", 14), ("v", 15)],
           [("q", 0, 1)], [("q", 0, 2)], [("q", 0, 3)],
           [("k", 1, 0)], [("k", 1, 1)], [("k", 1, 2)], [("k", 1, 3)],
           [("q", 1, 0)], [("q", 1, 1)], [("q", 1, 2)], [("q", 1, 3)]]
    aux.reverse()

    NG = SKT // 4

    def emit_pv(st):
        p, ch, g, eA, eB, ctx0, ctx1 = st
        for j in range(4):
            t = 4 * g + j
            js = slice(j * CW, (j + 1) * CW)
            nc.tensor.matmul(
                ctx0[:], vt[t][:, 2 * p, :], eA[:, js],
                start=(t == 0), stop=(t == SKT - 1),
            )
            nc.tensor.matmul(
                ctx1[:], vt[t][:, 2 * p + 1, :], eB[:, js],
                start=(t == 0), stop=(t == SKT - 1),
            )
        if g == NG - 1:
            sq = slice(ch * CW, (ch + 1) * CW)
            o0 = opool.tile([VW, CW], dt.float32, tag="o")
            o1 = opool.tile([VW, CW], dt.float32, tag="o")
            nc.vector.tensor_copy(o0[:], ctx0[:])
            nc.vector.tensor_copy(o1[:], ctx1[:])
            nc.sync.dma_start(out=ctxT[2 * p, :, sq], in_=o0[:])
            nc.sync.dma_start(out=ctxT[2 * p + 1, :, sq], in_=o1[:])

    pending = None
    for p in range(2):           # head pair: heads (2p, 2p+1)
        for ch in range(NCH):
            ctx0 = cpsum.tile([VW, CW], dt.float32, tag="ctx")
            ctx1 = cpsum.tile([VW, CW], dt.float32, tag="ctx")
            for g in range(NG):
                if aux:
                    for task in aux.pop():
                        if task[0] == "v":
                            proj_v_group(task[1])
                        else:
                            proj_qk_group(0 if task[0] == "q" else 1,
                                          task[1], task[2])
                psA = spsum.tile([P, 1024], dt.float32, tag="sc")
                psB = spsum.tile([P, 1024], dt.float32, tag="sc")
                qt = qT[p][ch // 2]
                qs = slice((ch % 2) * CW, (ch % 2) * CW + CW)
                for j in range(4):
                    t = 4 * g + j
                    kt = kT[p][g]
                    ks = slice(j * P, (j + 1) * P)
                    js = slice(j * CW, (j + 1) * CW)
                    # row-packed pair: PE rows 0-63 and 64-127
                    nc.tensor.matmul(
                        psA[:, js], kt[0:HD, ks], qt[0:HD, qs],
                        start=True, stop=True,
                    )
                    nc.tensor.matmul(
                        psB[:, js], kt[HD:P, ks], qt[HD:P, qs],
                        start=True, stop=True,
                    )
                eA = epool.tile([P, 1024], dt.float32r, tag="eA")
                eB = epool.tile([P, 1024], dt.float32r, tag="eB")
                nc.scalar.activation(eA[:], psA[:], AF.Exp, scale=0.125)
                nc.scalar.activation(eB[:], psB[:], AF.Exp, scale=0.125)
                if pending is not None:
                    emit_pv(pending)
                pending = (p, ch, g, eA, eB, ctx0, ctx1)
    emit_pv(pending)


def kernel(hidden_states, attention_mask, Wq, bq, Wk, bk, Wv, bv):
    hidden_states = np.asarray(hidden_states, dtype=np.float32)
    attention_mask = np.asarray(attention_mask, dtype=np.float32)
    Wq = np.asarray(Wq, dtype=np.float32)
    Wk = np.asarray(Wk, dtype=np.float32)
    Wv = np.asarray(Wv, dtype=np.float32)
    bq = np.asarray(bq, dtype=np.float32)
    bk = np.asarray(bk, dtype=np.float32)
    bv = np.asarray(bv, dtype=np.float32)

    import time as _time
    if "nc" not in _CACHE:
        _CACHE["nc"] = _build()
    nc = _CACHE["nc"]

    xTs = [np.ascontiguousarray(hidden_states[b].T) for b in range(B)]
    in_maps = []
    for c in range(8):
        b, g = c // 4, c % 4
        cols = slice(g * GW, (g + 1) * GW)
        in_maps.append({
            "xT": xTs[b],
            "wT": np.ascontiguousarray(
                np.stack([Wq[cols].T, Wk[cols].T, Wv[cols].T])),
            "bqk": np.ascontiguousarray(np.stack([bq[cols], bk[cols]])),
            "mask": np.ascontiguousarray(
                np.repeat(attention_mask[b, 0, 0, :, None], NH, axis=1)),
        })

    res = None
    for attempt in range(4):
        try:
            res = run_bass_kernel_spmd(nc, in_maps, list(range(8)))
            break
        except Exception:
            if attempt == 3:
                raise
            _time.sleep(90)  # transient axon mesh desyncs recover in ~2 min

    out = np.empty((B, S, D), dtype=np.float32)
    for c in range(8):
        b, g = c // 4, c % 4
        r = res.results[c]["ctxT"]                     # [NH, VW, S]
        ctx = r[:, :HD, :] / r[:, HD:HD + 1, :]        # [NH, HD, S]
        ctx = ctx + bv[g * GW:(g + 1) * GW].reshape(NH, HD)[:, :, None]
        out[b, :, g * GW:(g + 1) * GW] = ctx.transpose(2, 0, 1).reshape(S, GW)
    return out
